# revision 8
# baseline (speedup 1.0000x reference)
"""AVAlign kernel for 8 Trainium2 NeuronCores.

Sharding: data-parallel over segments, 4 segments per core (pairs (s, s^1)
stay on-core since 4 consecutive segs starting at a multiple of 4 contain
both members of each XOR pair).

Device computes (per core, float32r matmuls, fp32 accumulate):
  faT  [d=512, b=128]   : Wt-conv + max-pool over 8x8 (b = 4 segs x 32 cls)
  taT  [128, b=128]     : Wa @ fa + (ba + Wa bt)
  cvT  [hw=196, d=512]  : per video row, Ws-conv (bias folded out)
  FGT  [d=512, n=1024]  : cam-normalized foreground pool (n = 32 rows x 32 cls)
  tvT  [128, n=1024]    : Wv @ fg + (bv + Wv bs)
  G    [2, 4, 32, 256]  : ta . tv Gram vs own segment and partner segment

Host does input layout prep (transpose weights, normalize cam by its spatial
sum — exact algebra folds) and the final O(2048) selection:
  ||ta - tv||^2 = ||ta||^2 + ||tv||^2 - 2 G, gathered at (f*32+c) for 'same'
  pairs and (rand_frames*32+rand_classes) for 'differ' pairs, then masked.
"""

import numpy as np

S, FRAME, CLS, D, DOUT = 32, 8, 32, 512, 128
HWA, HWV = 64, 196          # flattened spatial dims (8*8, 14*14)
N_CORES = 8
S_PC = S // N_CORES         # 4 segments per core
BA = S_PC * CLS             # 128 audio rows per core
BV = S_PC * FRAME           # 32 video rows per core
NV = BV * CLS               # 1024 tv rows per core

_CACHE = {}


def _build_nc():
    from contextlib import ExitStack
    import concourse.bacc as bacc
    import concourse.tile as tile
    import concourse.mybir as mybir

    f32 = mybir.dt.float32
    f32r = mybir.dt.float32r
    AX = mybir.AxisListType.X

    nc = bacc.Bacc("TRN2", target_bir_lowering=False, debug=False,
                   enable_asserts=False, num_devices=N_CORES)

    feat_a = nc.dram_tensor("feat_a", [D, BA * HWA], f32r, kind="ExternalInput").ap()
    feat_v = nc.dram_tensor("feat_v", [D, BV * HWV], f32r, kind="ExternalInput").ap()
    cam_n = nc.dram_tensor("cam_n", [HWV, BV * CLS], f32r, kind="ExternalInput").ap()
    WtT = nc.dram_tensor("WtT", [D, D], f32r, kind="ExternalInput").ap()
    WsT = nc.dram_tensor("WsT", [D, D], f32r, kind="ExternalInput").ap()
    WaT = nc.dram_tensor("WaT", [D, DOUT], f32r, kind="ExternalInput").ap()
    WvT = nc.dram_tensor("WvT", [D, DOUT], f32r, kind="ExternalInput").ap()
    ba2 = nc.dram_tensor("ba2", [DOUT, 1], f32, kind="ExternalInput").ap()
    bv2 = nc.dram_tensor("bv2", [DOUT, 1], f32, kind="ExternalInput").ap()

    out_ta = nc.dram_tensor("out_ta", [DOUT, BA], f32r, kind="ExternalOutput").ap()
    out_tv = nc.dram_tensor("out_tv", [DOUT, NV], f32r, kind="ExternalOutput").ap()
    out_G = nc.dram_tensor("out_G", [2, S_PC, CLS, FRAME * CLS], f32,
                           kind="ExternalOutput").ap()

    HW_CHUNKS = [(0, 128), (128, HWV - 128)]   # 196 = 128 + 68

    with tile.TileContext(nc) as tc, ExitStack() as ctx:
        wpool = ctx.enter_context(tc.tile_pool(name="weights", bufs=1))
        persist = ctx.enter_context(tc.tile_pool(name="persist", bufs=1))
        apool = ctx.enter_context(tc.tile_pool(name="apool", bufs=3))
        bpool = ctx.enter_context(tc.tile_pool(name="bpool", bufs=2))
        gpool = ctx.enter_context(tc.tile_pool(name="gpool", bufs=2))
        ps_a = ctx.enter_context(tc.tile_pool(name="ps_a", bufs=2, space="PSUM"))
        ps_cv = ctx.enter_context(tc.tile_pool(name="ps_cv", bufs=2, space="PSUM"))
        ps_fg = ctx.enter_context(tc.tile_pool(name="ps_fg", bufs=1, space="PSUM"))
        ps_m = ctx.enter_context(tc.tile_pool(name="ps_m", bufs=1, space="PSUM"))

        # ---- replicated weights into SBUF (Wt first; rest deferred) ----
        wt_sb, ws_sb, wa_sb, wv_sb = [], [], [], []
        for ic in range(4):
            t = wpool.tile([128, D], f32r, tag=f"wt{ic}", name=f"wt{ic}")
            nc.sync.dma_start(t[:], WtT[ic * 128:(ic + 1) * 128, :])
            wt_sb.append(t)

        def emit_late_weights():
            for ic in range(4):
                t = wpool.tile([128, D], f32r, tag=f"ws{ic}", name=f"ws{ic}")
                nc.sync.dma_start(t[:], WsT[ic * 128:(ic + 1) * 128, :])
                ws_sb.append(t)
                t = wpool.tile([128, DOUT], f32r, tag=f"wa{ic}", name=f"wa{ic}")
                nc.sync.dma_start(t[:], WaT[ic * 128:(ic + 1) * 128, :])
                wa_sb.append(t)
                t = wpool.tile([128, DOUT], f32r, tag=f"wv{ic}", name=f"wv{ic}")
                nc.sync.dma_start(t[:], WvT[ic * 128:(ic + 1) * 128, :])
                wv_sb.append(t)
            t = wpool.tile([DOUT, 1], f32, tag="ba2", name="ba2sb")
            nc.sync.dma_start(t[:], ba2[:, :])
            misc_sb.append(t)
            t = wpool.tile([DOUT, 1], f32, tag="bv2", name="bv2sb")
            nc.sync.dma_start(t[:], bv2[:, :])
            misc_sb.append(t)
        misc_sb = []

        faT = [persist.tile([128, BA], f32r, tag=f"faT{oc}", name=f"faT{oc}") for oc in range(4)]
        cam_sb = []

        def emit_cam_load():
            for hc, (h0, hn) in enumerate(HW_CHUNKS):
                t = persist.tile([hn, BV * CLS], f32r, tag=f"camA{hc}", name=f"camA{hc}")
                nc.sync.dma_start(t[:], cam_n[h0:h0 + hn, :])
                cam_sb.append(t)
        FGT = [persist.tile([128, NV], f32r, tag=f"FGT{dc}", name=f"FGT{dc}") for dc in range(4)]
        taT = persist.tile([DOUT, BA], f32r, tag="taT", name="taT")
        tvT = persist.tile([DOUT, NV], f32r, tag="tvT", name="tvT")

        # ---- stage A: feat_a -> faT (conv + maxpool), 16-b groups ----
        GA = 16 * HWA  # 1024 cols per group DMA, two 512-wide matmul halves

        def emit_a_group(ag):
            rhs = []
            for ic in range(4):
                t = apool.tile([128, GA], f32r, tag=f"rhsA{ic}", name=f"rhsA{ic}")
                nc.sync.dma_start(
                    t[:], feat_a[ic * 128:(ic + 1) * 128,
                                 ag * GA:(ag + 1) * GA])
                rhs.append(t)
            for oc in range(4):
                for half in range(2):
                    ps = ps_a.tile([128, 8 * HWA], f32, tag="psA", name="psA")
                    for ic in range(4):
                        nc.tensor.matmul(
                            ps[:],
                            wt_sb[ic][:, oc * 128:(oc + 1) * 128],
                            rhs[ic][:, half * 512:(half + 1) * 512],
                            start=(ic == 0), stop=(ic == 3))
                    nc.vector.reduce_max(
                        faT[oc][:, ag * 16 + half * 8:ag * 16 + half * 8 + 8],
                        ps[:].rearrange("p (b h) -> p b h", h=HWA),
                        axis=AX)

        # ---- stage B: per video row: cvT then FGT columns ----
        GV = 4 * HWV  # 4 video rows per feat_v DMA group
        fvg = [None]

        def emit_b_row(b):
            j = b % 4
            if j == 0:
                g = b // 4
                fvg[0] = []
                for ic in range(4):
                    t = bpool.tile([128, GV], f32r, tag=f"fvb{ic}", name=f"fvb{ic}")
                    nc.sync.dma_start(
                        t[:], feat_v[ic * 128:(ic + 1) * 128,
                                     g * GV:(g + 1) * GV])
                    fvg[0].append(t)
            fvb = fvg[0]
            cvt = []
            for hc, (h0, hn) in enumerate(HW_CHUNKS):
                ps = ps_cv.tile([hn, D], f32, tag=f"psCV{hc}", name=f"psCV{hc}")
                for ic in range(4):
                    nc.tensor.matmul(
                        ps[:],
                        fvb[ic][:, j * HWV + h0:j * HWV + h0 + hn],
                        ws_sb[ic][:],
                        start=(ic == 0), stop=(ic == 3))
                t = bpool.tile([hn, D], f32r, tag=f"cvT{hc}", name=f"cvT{hc}")
                nc.any.tensor_copy(t[:], ps[:])
                cvt.append(t)
            pfg = ps_fg.tile([128, 4 * CLS], f32, tag="psFG", name="psFG")
            for dc in range(4):
                for hc, (h0, hn) in enumerate(HW_CHUNKS):
                    nc.tensor.matmul(
                        pfg[:, dc * CLS:(dc + 1) * CLS],
                        cvt[hc][:, dc * 128:(dc + 1) * 128],
                        cam_sb[hc][:, b * CLS:(b + 1) * CLS],
                        start=(hc == 0), stop=(hc == 1))
            for dc in range(4):
                nc.any.tensor_copy(FGT[dc][:, b * CLS:(b + 1) * CLS],
                                   pfg[:, dc * CLS:(dc + 1) * CLS])

        # interleave A and B groups so PE alternates independent streams;
        # first A group's DMA goes out before the stage-B weights/cam so the
        # PE starts as early as possible
        emit_a_group(0)
        emit_late_weights()
        emit_cam_load()
        for b in range(0, 4):
            emit_b_row(b)
        for i in range(1, 8):
            emit_a_group(i)
            for b in range(i * 4, (i + 1) * 4):
                emit_b_row(b)

        # ---- taT = WaT.T @ faT + ba2 ----
        pta = ps_m.tile([DOUT, BA], f32, tag="psM", name="psM")
        for dc in range(4):
            nc.tensor.matmul(pta[:], wa_sb[dc][:],
                             faT[dc][:],
                             start=(dc == 0), stop=(dc == 3))
        nc.vector.tensor_scalar_add(taT[:], pta[:], misc_sb[0][:, 0:1])
        nc.sync.dma_start(out_ta[:, :], taT[:])

        # ---- tvT = WvT.T @ FGT + bv2 ----
        for ng in range(NV // 512):
            ptv = ps_m.tile([DOUT, 512], f32, tag="psM", name="psM")
            for dc in range(4):
                nc.tensor.matmul(
                    ptv[:], wv_sb[dc][:],
                    FGT[dc][:, ng * 512:(ng + 1) * 512],
                    start=(dc == 0), stop=(dc == 3))
            nc.vector.tensor_scalar_add(tvT[:, ng * 512:(ng + 1) * 512],
                                        ptv[:], misc_sb[1][:, 0:1])
        nc.sync.dma_start(out_tv[:, :], tvT[:])

        # ---- Gram: G[which][s] = ta[s].T @ tv[s or s^1] ----
        NB = FRAME * CLS  # 256 tv rows per segment
        for s in range(S_PC):
            for which, sp in ((0, s), (1, s ^ 1)):
                psg = ps_m.tile([CLS, NB], f32, tag="psM", name="psM")
                nc.tensor.matmul(
                    psg[:],
                    taT[:, s * CLS:(s + 1) * CLS],
                    tvT[:, sp * NB:(sp + 1) * NB],
                    start=True, stop=True)
                g_sb = gpool.tile([CLS, NB], f32, tag="gsb", name="gsb")
                nc.any.tensor_copy(g_sb[:], psg[:])
                nc.sync.dma_start(out_G[which, s], g_sb[:])

    nc.compile()
    return nc


def _get_nc():
    if "nc" not in _CACHE:
        _CACHE["nc"] = _build_nc()
    return _CACHE["nc"]


def _prep_in_maps(inputs):
    feat_a = np.ascontiguousarray(
        np.asarray(inputs["feat_a"], np.float32)
        .reshape(N_CORES, BA, D, HWA).transpose(0, 2, 1, 3)
        .reshape(N_CORES, D, BA * HWA))
    feat_v = np.ascontiguousarray(
        np.asarray(inputs["feat_v"], np.float32)
        .reshape(N_CORES, BV, D, HWV).transpose(0, 2, 1, 3)
        .reshape(N_CORES, D, BV * HWV))
    cam = np.asarray(inputs["cam"], np.float32).reshape(S * FRAME, CLS, HWV)
    cam_n = np.ascontiguousarray(
        (cam / (cam.sum(-1, keepdims=True) + 1e-10))
        .reshape(N_CORES, BV, CLS, HWV).transpose(0, 3, 1, 2)
        .reshape(N_CORES, HWV, BV * CLS))
    Wt = np.asarray(inputs["Wt"], np.float32)
    Ws = np.asarray(inputs["Ws"], np.float32)
    Wa = np.asarray(inputs["Wa"], np.float32)
    Wv = np.asarray(inputs["Wv"], np.float32)
    bt = np.asarray(inputs["bt"], np.float32)
    bs = np.asarray(inputs["bs"], np.float32)
    ba = np.asarray(inputs["ba"], np.float32)
    bv = np.asarray(inputs["bv"], np.float32)
    shared = {
        "WtT": np.ascontiguousarray(Wt.T),
        "WsT": np.ascontiguousarray(Ws.T),
        "WaT": np.ascontiguousarray(Wa.T),
        "WvT": np.ascontiguousarray(Wv.T),
        "ba2": np.ascontiguousarray((ba + Wa @ bt).reshape(DOUT, 1)),
        "bv2": np.ascontiguousarray((bv + Wv @ bs).reshape(DOUT, 1)),
    }
    in_maps = []
    for k in range(N_CORES):
        m = dict(shared)
        m["feat_a"] = feat_a[k]
        m["feat_v"] = feat_v[k]
        m["cam_n"] = cam_n[k]
        in_maps.append(m)
    return in_maps


def _assemble(inputs, results):
    ta = np.concatenate(
        [r["out_ta"].T.reshape(S_PC, CLS, DOUT) for r in results])   # [S,C,O]
    tv = np.concatenate(
        [r["out_tv"].T.reshape(S_PC, FRAME, CLS, DOUT) for r in results])
    Gself = np.concatenate([r["out_G"][0] for r in results])         # [S,C,256]
    Gcross = np.concatenate([r["out_G"][1] for r in results])        # [S,C,256]

    tan = np.einsum('sco,sco->sc', ta, ta)                   # [S, C]
    tvn = np.einsum('sfco,sfco->sfc', tv, tv)                # [S, F, C]

    pred_a = np.asarray(inputs["pred_a"], np.float32)
    pred_v = np.asarray(inputs["pred_v"], np.float32)
    rf = np.asarray(inputs["rand_frames"])
    rc = np.asarray(inputs["rand_classes"])

    pv = 1.0 / (1.0 + np.exp(-pred_v.reshape(S, FRAME, CLS)))
    active_a = pred_a > 0.3
    active_v = pv > 0.3
    c_idx = np.arange(CLS)
    f_idx = np.arange(FRAME)

    G4 = Gself.reshape(S, CLS, FRAME, CLS)
    Gco = G4[:, c_idx[:, None], f_idx[None, :], c_idx[:, None]]  # [S, C, F]
    mask_co = active_a[:, :, None] & active_v.transpose(0, 2, 1)
    loss_co = (tan[:, :, None] + tvn.transpose(0, 2, 1) - 2.0 * Gco) / DOUT
    loss_co = loss_co * mask_co

    j = rf * CLS + rc                                        # [S, C, F]
    Gdi = np.take_along_axis(Gcross, j.reshape(S, CLS, FRAME), axis=2)
    tvn_p = tvn.reshape(S, FRAME * CLS)[np.arange(S) ^ 1]    # partner norms
    tvn_di = np.take_along_axis(tvn_p[:, None, :].repeat(CLS, 1),
                                j.reshape(S, CLS, FRAME), axis=2)
    num = (pred_a * FRAME).astype(np.int32)
    mask_di = active_a[:, :, None] & (f_idx[None, None, :] < num[:, :, None])
    loss_di = (tan[:, :, None] + tvn_di - 2.0 * Gdi) / DOUT
    loss_di = loss_di * mask_di

    return np.stack([loss_co, loss_di]).astype(np.float32)   # [2, S, C, F]


def _run(inputs, trace=False):
    from concourse.bass_utils import run_bass_kernel_spmd
    nc = _get_nc()
    in_maps = _prep_in_maps(inputs)
    try:
        br = run_bass_kernel_spmd(nc, in_maps, list(range(N_CORES)), trace=trace)
    except ModuleNotFoundError:
        br = run_bass_kernel_spmd(nc, in_maps, list(range(N_CORES)), trace=False)
    return _assemble(inputs, br.results), br


def kernel(**inputs):
    out, _ = _run(inputs)
    return out
